# revision 11
# baseline (speedup 1.0000x reference)
"""CrossAttention TRN2 Bass kernel.

Problem: out[b] = softmax((q[b] @ Wq.T) @ (k[b] @ Wk.T).T) @ (v[b] @ Wv.T)
  q/k/v: [8, 2048, 512] f32, Wq/Wk/Wv: [512, 512] f32.

Sharding: data-parallel over batch -- core b computes batch b entirely.

Key optimizations vs the reference structure:
  * All operand transposes happen on the HOST (numpy) -- the device receives
    qT/kT/vT [D, N] and WqT/WkT/WvT [D, D], so the PE spends zero cycles
    transposing inputs.
  * Every projection / score matmul runs in f32r (fp32 bits, PE rounds
    operands to ~11-bit mantissa, 1 cycle/col vs fp32's 4).  Simulated
    end-to-end rel-err of this scheme is ~1.5e-2 against the 2e-2 gate.
  * Softmax weights are written as bf16 by the ACT exp, transposed on the PE
    at 1 cycle/col (vs 2 for fp32), and the output matmul runs bf16xbf16.
  * Phase C is software-pipelined: scores(ib+1) is issued to the PE between
    exp(ib) (ACT) and the weight-transposes(ib), so the PE never waits on
    the softmax statistics chain.

Per-core PE budget @2.4GHz: k'/q' proj 66k cyc, v' proj 33k, scores 131k,
w transposes 33k, output 131k  ->  ~165us + DMA lead-in.
"""
import sys

if "/opt/trn_rl_repo" not in sys.path:
    sys.path.insert(0, "/opt/trn_rl_repo")

import numpy as np

import concourse.bacc as bacc
import concourse.mybir as mybir
import concourse.tile as tile
from concourse.bass_utils import run_bass_kernel_spmd
from concourse.masks import make_identity

F32 = mybir.dt.float32
F32R = mybir.dt.float32r
BF16 = mybir.dt.bfloat16
AX = mybir.AxisListType.X
ALU = mybir.AluOpType
EXP = mybir.ActivationFunctionType.Exp

B, NQ, NK, D = 8, 2048, 2048, 512
P = 128
NDB = D // P    # feature blocks (4)
NIB = NQ // P   # query row blocks (16)
NJB = NK // P   # key row blocks (16)
JC = 512        # scores j-chunk width (one fp32 PSUM bank)
NJC = NK // JC  # 4

_CACHE = {}


def _build():
    nc = bacc.Bacc("TRN2", target_bir_lowering=False)
    qT_d = nc.dram_tensor("qT", [D, NQ], F32R, kind="ExternalInput")
    kT_d = nc.dram_tensor("kT", [D, NK], F32R, kind="ExternalInput")
    vT_d = nc.dram_tensor("vT", [D, NK], F32R, kind="ExternalInput")
    w_d = {
        # wq/wk NATIVE [e, d] (feed MT = Wk^T Wq contraction over e);
        # wv transposed [d, d'] (rhs of the v' projection)
        "wq": nc.dram_tensor("wqN", [D, D], F32R, kind="ExternalInput"),
        "wk": nc.dram_tensor("wkN", [D, D], F32R, kind="ExternalInput"),
        "wv": nc.dram_tensor("wvT", [D, D], F32R, kind="ExternalInput"),
    }
    out_d = nc.dram_tensor("out", [NQ, D], F32, kind="ExternalOutput")

    with tile.TileContext(nc) as tc:
        with tc.tile_pool(name="persist", bufs=1) as pp:
            ident_b = pp.tile([P, P], BF16, tag="ident_b")
            make_identity(nc, ident_b[:])

            # persistent across phase C: raw qT chunks (scores lhsT), the
            # folded Mk = (Wq^T Wk) k^T (scores rhs), and v'
            qt = [pp.tile([P, NDB, JC], F32R, tag=f"qt{c}", name=f"qt{c}") for c in range(NJC)]
            mk = [pp.tile([P, NK], F32R, tag=f"mk{db}", name=f"mk{db}") for db in range(NDB)]
            vp = [pp.tile([P, D], BF16, tag=f"vp{jb}", name=f"vp{jb}") for jb in range(NJB)]

            # ---------------- Phase B (PE order: MT, Mk, v')
            # scores = q' k'^T = q (Wq^T Wk) k^T: fold the two projection
            # weights into M once (tiny), apply M to k^T only, and feed raw
            # qT as the scores stationary -- deletes the whole q' projection.
            with (
                tc.tile_pool(name="wp", bufs=1) as wp,
                tc.tile_pool(name="xp", bufs=1) as xp,
                tc.tile_pool(name="psP", bufs=4, space="PSUM") as psP,
            ):
                # Critical path on the SP (sync) hwdge queue: Wk, Wq, kT.
                # Everything else (Wv, vT, qT) on the ACT (scalar) queue.
                def wtile(wname, eng):
                    t = wp.tile([P, NDB, D], F32R, tag=f"wt_{wname}", name=f"wt_{wname}")
                    eng.dma_start(t[:], w_d[wname].rearrange("(a p) e -> p a e", p=P))
                    return t

                def load_x(xd, tiles, eng):
                    xre = xd.rearrange("(a p) n -> p a n", p=P)
                    for c in range(NJC):
                        sl = slice(c * JC, (c + 1) * JC)
                        eng.dma_start(tiles[c][:], xre[:, :, sl])
                    return tiles

                # DMA emission is interleaved with consumers: the tile
                # scheduler ties sequence points to emission order, so a
                # consumer emitted after N dma_starts can end up gated on
                # transfers it never reads.  Emit each load right before
                # its first consumer.
                wt = {}
                wt["wk"] = wtile("wk", nc.sync)
                wt["wq"] = wtile("wq", nc.sync)

                # MT[d2, d1] = sum_e Wk[e, d2] Wq[e, d1]  (16 matmuls)
                mtt = [wp.tile([P, D], F32R, tag=f"mtt{b2}", name=f"mtt{b2}") for b2 in range(NDB)]
                for b2 in range(NDB):
                    pm = psP.tile([P, D], F32, tag="pm")
                    for a in range(NDB):
                        nc.tensor.matmul(
                            pm[:],
                            wt["wk"][:, a, b2 * P : (b2 + 1) * P],
                            wt["wq"][:, a, :],
                            start=(a == 0),
                            stop=(a == NDB - 1),
                        )
                    nc.any.tensor_copy(mtt[b2][:], pm[:])

                # Mk[d1, j] = sum_d2 MT[d2, d1] kT[d2, j]  (64 matmuls),
                # kT chunk DMA emitted just ahead of its consumer chunk
                kre = kT_d.rearrange("(a p) n -> p a n", p=P)
                for c in range(NJC):
                    sl = slice(c * JC, (c + 1) * JC)
                    ktc = xp.tile([P, NDB, JC], F32R, tag=f"kt{c}", name=f"kt{c}")
                    nc.sync.dma_start(ktc[:], kre[:, :, sl])
                    for b1 in range(NDB):
                        pm = psP.tile([P, JC], F32, tag="pm")
                        for b2 in range(NDB):
                            nc.tensor.matmul(
                                pm[:],
                                mtt[b2][:, b1 * P : (b1 + 1) * P],
                                ktc[:, b2, :],
                                start=(b2 == 0),
                                stop=(b2 == NDB - 1),
                            )
                        nc.any.tensor_copy(mk[b1][:, sl], pm[:])

                # v' projection: v'[j, d'] = sum_d vT[d, j] WvT[d, d']
                wt["wv"] = wtile("wv", nc.scalar)
                vre = vT_d.rearrange("(a p) n -> p a n", p=P)
                for jb in range(NJB):
                    if jb % 4 == 0:
                        vtc = xp.tile([P, NDB, JC], F32R, tag=f"vt{jb // 4}", name=f"vt{jb // 4}")
                        nc.scalar.dma_start(vtc[:], vre[:, :, (jb // 4) * JC : (jb // 4 + 1) * JC])
                    pm = psP.tile([P, D], F32, tag="pm")
                    for db in range(NDB):
                        nc.tensor.matmul(
                            pm[:],
                            vtc[:, db, (jb % 4) * P : (jb % 4 + 1) * P],
                            wt["wv"][:, db, :],
                            start=(db == 0),
                            stop=(db == NDB - 1),
                        )
                    nc.any.tensor_copy(vp[jb][:], pm[:])

                load_x(qT_d, qt, nc.scalar)

            # ---------------- Phase C: attention, pipelined over query blocks
            with (
                tc.tile_pool(name="cs", bufs=2) as cs,
                tc.tile_pool(name="stat", bufs=2) as st,
                tc.tile_pool(name="psS", bufs=5, space="PSUM") as psS,
                tc.tile_pool(name="psT", bufs=2, space="PSUM") as psT,
                tc.tile_pool(name="psO", bufs=1, space="PSUM") as psO,
            ):
                def emit_scores(ib):
                    qtile = qt[ib // 4]
                    io = (ib % 4) * P
                    chunks = []
                    for c in range(NJC):
                        jsl = slice(c * JC, (c + 1) * JC)
                        sc_ = psS.tile([P, JC], F32, tag="sc")
                        for b1 in range(NDB):
                            nc.tensor.matmul(
                                sc_[:],
                                qtile[:, b1, io : io + P],
                                mk[b1][:, jsl],
                                start=(b1 == 0),
                                stop=(b1 == NDB - 1),
                            )
                        chunks.append(sc_)
                    return chunks

                sch = emit_scores(0)
                for ib in range(NIB):
                    cur = sch
                    # --- softmax stats (DVE) + exp (ACT) for ib
                    nmax = []
                    for c in range(NJC):
                        nm = st.tile([P, 1], F32, tag=f"nm{c}", name=f"nm{c}")
                        nc.vector.reduce_max(nm[:], cur[c][:], axis=AX, negate=True)
                        nmax.append(nm)
                    nm01 = st.tile([P, 1], F32, tag="nm01")
                    nc.vector.tensor_tensor(nm01[:], nmax[0][:], nmax[1][:], op=ALU.min)
                    nm23 = st.tile([P, 1], F32, tag="nm23")
                    nc.vector.tensor_tensor(nm23[:], nmax[2][:], nmax[3][:], op=ALU.min)
                    nmall = st.tile([P, 1], F32, tag="nmall")
                    nc.vector.tensor_tensor(nmall[:], nm01[:], nm23[:], op=ALU.min)

                    w_sb = cs.tile([P, NK], BF16, tag="w")
                    dcs = []
                    for c in range(NJC):
                        dc = st.tile([P, 1], F32, tag=f"dc{c}", name=f"dc{c}")
                        nc.scalar.activation(
                            w_sb[:, c * JC : (c + 1) * JC],
                            cur[c][:],
                            EXP,
                            bias=nmall[:],
                            scale=1.0,
                            accum_out=dc[:],
                        )
                        dcs.append(dc)
                    d01 = st.tile([P, 1], F32, tag="d01")
                    nc.vector.tensor_tensor(d01[:], dcs[0][:], dcs[1][:], op=ALU.add)
                    d23 = st.tile([P, 1], F32, tag="d23")
                    nc.vector.tensor_tensor(d23[:], dcs[2][:], dcs[3][:], op=ALU.add)
                    den = st.tile([P, 1], F32, tag="den")
                    nc.vector.tensor_tensor(den[:], d01[:], d23[:], op=ALU.add)
                    rinv = st.tile([P, 1], F32, tag="rinv")
                    nc.vector.reciprocal(rinv[:], den[:])

                    # --- next block's scores keep the PE busy during exp(ib)
                    if ib + 1 < NIB:
                        sch = emit_scores(ib + 1)

                    # --- transpose exp weights (bf16, 1 cyc/col); bf16 PSUM
                    # tiles pack 8 transposes per bank and give 2x DVE copies
                    wT = cs.tile([P, NK], BF16, tag="wT")
                    for g in range(2):
                        pt = psT.tile([P, 8 * P], BF16, tag="pt")
                        for jj in range(8):
                            js = 8 * g + jj
                            nc.tensor.transpose(
                                pt[:, jj * P : (jj + 1) * P],
                                w_sb[:, js * P : (js + 1) * P],
                                ident_b[:],
                            )
                        nc.vector.tensor_copy(wT[:, g * 8 * P : (g + 1) * 8 * P], pt[:])

                    # --- output matmul
                    po = psO.tile([P, D], F32, tag="po")
                    for js in range(NJB):
                        nc.tensor.matmul(
                            po[:],
                            wT[:, js * P : (js + 1) * P],
                            vp[js][:],
                            start=(js == 0),
                            stop=(js == NJB - 1),
                        )
                    ob = cs.tile([P, D], F32, tag="ob")
                    nc.vector.tensor_scalar_mul(ob[:], po[:], rinv[:])
                    nc.sync.dma_start(out_d[ib * P : (ib + 1) * P, :], ob[:])

    nc.compile()
    return nc


def _get_nc():
    if "nc" not in _CACHE:
        _CACHE["nc"] = _build()
    return _CACHE["nc"]


def kernel(query, key, value, Wq, Wk, Wv, _trace=False):
    query = np.asarray(query, dtype=np.float32)
    key = np.asarray(key, dtype=np.float32)
    value = np.asarray(value, dtype=np.float32)
    qT = np.ascontiguousarray(query.transpose(0, 2, 1))
    kT = np.ascontiguousarray(key.transpose(0, 2, 1))
    vT = np.ascontiguousarray(value.transpose(0, 2, 1))
    wqN = np.ascontiguousarray(np.asarray(Wq, dtype=np.float32))
    wkN = np.ascontiguousarray(np.asarray(Wk, dtype=np.float32))
    wvT = np.ascontiguousarray(np.asarray(Wv, dtype=np.float32).T)

    nc = _get_nc()
    in_maps = [
        {
            "qT": qT[b],
            "kT": kT[b],
            "vT": vT[b],
            "wqN": wqN,
            "wkN": wkN,
            "wvT": wvT,
        }
        for b in range(B)
    ]
    res = run_bass_kernel_spmd(nc, in_maps, list(range(B)), trace=_trace)
    out = np.stack([res.results[b]["out"] for b in range(B)]).astype(np.float32)
    if _trace:
        _CACHE["last_result"] = res
    return out


# revision 15
# speedup vs baseline: 1.0094x; 1.0094x over previous
"""CrossAttention TRN2 Bass kernel.

Problem: out[b] = softmax((q[b] @ Wq.T) @ (k[b] @ Wk.T).T) @ (v[b] @ Wv.T)
  q/k/v: [8, 2048, 512] f32, Wq/Wk/Wv: [512, 512] f32.

Sharding: data-parallel over batch -- core b computes batch b entirely.

Key optimizations vs the reference structure:
  * All operand transposes happen on the HOST (numpy) -- the device receives
    qT/kT/vT [D, N] and WqT/WkT/WvT [D, D], so the PE spends zero cycles
    transposing inputs.
  * Every projection / score matmul runs in f32r (fp32 bits, PE rounds
    operands to ~11-bit mantissa, 1 cycle/col vs fp32's 4).  Simulated
    end-to-end rel-err of this scheme is ~1.5e-2 against the 2e-2 gate.
  * Softmax weights are written as bf16 by the ACT exp, transposed on the PE
    at 1 cycle/col (vs 2 for fp32), and the output matmul runs bf16xbf16.
  * Phase C is software-pipelined: scores(ib+1) is issued to the PE between
    exp(ib) (ACT) and the weight-transposes(ib), so the PE never waits on
    the softmax statistics chain.

Per-core PE budget @2.4GHz: k'/q' proj 66k cyc, v' proj 33k, scores 131k,
w transposes 33k, output 131k  ->  ~165us + DMA lead-in.
"""
import sys

if "/opt/trn_rl_repo" not in sys.path:
    sys.path.insert(0, "/opt/trn_rl_repo")

import numpy as np

import concourse.bacc as bacc
import concourse.mybir as mybir
import concourse.tile as tile
from concourse.bass_utils import run_bass_kernel_spmd
from concourse.masks import make_identity

F32 = mybir.dt.float32
F32R = mybir.dt.float32r
BF16 = mybir.dt.bfloat16
AX = mybir.AxisListType.X
ALU = mybir.AluOpType
EXP = mybir.ActivationFunctionType.Exp

B, NQ, NK, D = 8, 2048, 2048, 512
P = 128
NDB = D // P    # feature blocks (4)
NIB = NQ // P   # query row blocks (16)
NJB = NK // P   # key row blocks (16)
JC = 512        # scores j-chunk width (one fp32 PSUM bank)
NJC = NK // JC  # 4

_CACHE = {}


def _build():
    nc = bacc.Bacc("TRN2", target_bir_lowering=False)
    qT_d = nc.dram_tensor("qT", [D, NQ], F32R, kind="ExternalInput")
    kT_d = nc.dram_tensor("kT", [D, NK], F32R, kind="ExternalInput")
    # v path is bf16 end-to-end: v' only needs ~8 bits (it lands in bf16
    # anyway) and halving vT's bytes relieves the shared DMA engine
    vT_d = nc.dram_tensor("vT", [D, NK], BF16, kind="ExternalInput")
    w_d = {
        # wq/wk NATIVE [e, d] (feed MT = Wk^T Wq contraction over e);
        # wv transposed [d, d'] (rhs of the v' projection)
        "wq": nc.dram_tensor("wqN", [D, D], F32R, kind="ExternalInput"),
        "wk": nc.dram_tensor("wkN", [D, D], F32R, kind="ExternalInput"),
        "wv": nc.dram_tensor("wvT", [D, D], BF16, kind="ExternalInput"),
    }
    out_d = nc.dram_tensor("out", [NQ, D], F32, kind="ExternalOutput")

    with tile.TileContext(nc) as tc:
        with tc.tile_pool(name="persist", bufs=1) as pp:
            ident_b = pp.tile([P, P], BF16, tag="ident_b")
            make_identity(nc, ident_b[:])

            # persistent across phase C: raw qT chunks (scores lhsT), the
            # folded Mk = (Wq^T Wk) k^T (scores rhs), and v'
            qt = [pp.tile([P, NDB, JC], F32R, tag=f"qt{c}", name=f"qt{c}") for c in range(NJC)]
            mk = [pp.tile([P, NK], F32R, tag=f"mk{db}", name=f"mk{db}") for db in range(NDB)]
            vp = [pp.tile([P, D], BF16, tag=f"vp{jb}", name=f"vp{jb}") for jb in range(NJB)]

            # ---------------- Phase B (PE order: MT, Mk, v')
            # scores = q' k'^T = q (Wq^T Wk) k^T: fold the two projection
            # weights into M once (tiny), apply M to k^T only, and feed raw
            # qT as the scores stationary -- deletes the whole q' projection.
            with (
                tc.tile_pool(name="wp", bufs=1) as wp,
                tc.tile_pool(name="xp", bufs=1) as xp,
                tc.tile_pool(name="psP", bufs=4, space="PSUM") as psP,
            ):
                # Critical path on the SP (sync) hwdge queue: Wk, Wq, kT.
                # Everything else (Wv, vT, qT) on the ACT (scalar) queue.
                def wtile(wname, eng):
                    t = wp.tile([P, NDB, D], F32R, tag=f"wt_{wname}", name=f"wt_{wname}")
                    eng.dma_start(t[:], w_d[wname].rearrange("(a p) e -> p a e", p=P))
                    return t

                def load_x(xd, tiles, eng):
                    xre = xd.rearrange("(a p) n -> p a n", p=P)
                    for c in range(NJC):
                        sl = slice(c * JC, (c + 1) * JC)
                        eng.dma_start(tiles[c][:], xre[:, :, sl])
                    return tiles

                # All DMA issue up front: the issuing engine queues are
                # in-order, so any compute assigned to ACT/SP later must not
                # sit ahead of a dma_start (it would delay the transfer).
                # Critical k/q-fold path on the SP queue, v/q loads on the
                # ACT queue.
                wt = {}
                wt["wk"] = wtile("wk", nc.sync)
                wt["wq"] = wtile("wq", nc.sync)
                kre = kT_d.rearrange("(a p) n -> p a n", p=P)
                kt = []
                for c in range(NJC):
                    ktc = xp.tile([P, NDB, JC], F32R, tag=f"kt{c}", name=f"kt{c}")
                    nc.sync.dma_start(ktc[:], kre[:, :, c * JC : (c + 1) * JC])
                    kt.append(ktc)
                wt["wv"] = wp.tile([P, NDB, D], BF16, tag="wt_wv", name="wt_wv")
                nc.scalar.dma_start(wt["wv"][:], w_d["wv"].rearrange("(a p) e -> p a e", p=P))
                vre = vT_d.rearrange("(a p) n -> p a n", p=P)
                vt = []
                for c in range(NJC):
                    vtc = xp.tile([P, NDB, JC], BF16, tag=f"vt{c}", name=f"vt{c}")
                    nc.scalar.dma_start(vtc[:], vre[:, :, c * JC : (c + 1) * JC])
                    vt.append(vtc)
                load_x(qT_d, qt, nc.scalar)

                # MT[d2, d1] = sum_e Wk[e, d2] Wq[e, d1]  (16 matmuls)
                mtt = [wp.tile([P, D], F32R, tag=f"mtt{b2}", name=f"mtt{b2}") for b2 in range(NDB)]
                for b2 in range(NDB):
                    pm = psP.tile([P, D], F32, tag="pm")
                    for a in range(NDB):
                        nc.tensor.matmul(
                            pm[:],
                            wt["wk"][:, a, b2 * P : (b2 + 1) * P],
                            wt["wq"][:, a, :],
                            start=(a == 0),
                            stop=(a == NDB - 1),
                        )
                    nc.vector.tensor_copy(mtt[b2][:], pm[:])

                # Mk[d1, j] = sum_d2 MT[d2, d1] kT[d2, j]  (64 matmuls)
                for c in range(NJC):
                    sl = slice(c * JC, (c + 1) * JC)
                    for b1 in range(NDB):
                        pm = psP.tile([P, JC], F32, tag="pm")
                        for b2 in range(NDB):
                            nc.tensor.matmul(
                                pm[:],
                                mtt[b2][:, b1 * P : (b1 + 1) * P],
                                kt[c][:, b2, :],
                                start=(b2 == 0),
                                stop=(b2 == NDB - 1),
                            )
                        nc.vector.tensor_copy(mk[b1][:, sl], pm[:])

                # v' projection: v'[j, d'] = sum_d vT[d, j] WvT[d, d']
                for jb in range(NJB):
                    pm = psP.tile([P, D], F32, tag="pm")
                    for db in range(NDB):
                        nc.tensor.matmul(
                            pm[:],
                            vt[jb // 4][:, db, (jb % 4) * P : (jb % 4 + 1) * P],
                            wt["wv"][:, db, :],
                            start=(db == 0),
                            stop=(db == NDB - 1),
                        )
                    nc.vector.tensor_copy(vp[jb][:], pm[:])

            # ---------------- Phase C: attention, pipelined over query blocks
            with (
                tc.tile_pool(name="cs", bufs=2) as cs,
                tc.tile_pool(name="stat", bufs=2) as st,
                tc.tile_pool(name="psS", bufs=5, space="PSUM") as psS,
                tc.tile_pool(name="psT", bufs=2, space="PSUM") as psT,
                tc.tile_pool(name="psO", bufs=1, space="PSUM") as psO,
            ):
                def emit_scores(ib):
                    qtile = qt[ib // 4]
                    io = (ib % 4) * P
                    chunks = []
                    for c in range(NJC):
                        jsl = slice(c * JC, (c + 1) * JC)
                        sc_ = psS.tile([P, JC], F32, tag="sc")
                        for b1 in range(NDB):
                            nc.tensor.matmul(
                                sc_[:],
                                qtile[:, b1, io : io + P],
                                mk[b1][:, jsl],
                                start=(b1 == 0),
                                stop=(b1 == NDB - 1),
                            )
                        chunks.append(sc_)
                    return chunks

                sch = emit_scores(0)
                for ib in range(NIB):
                    cur = sch
                    # --- softmax stats (DVE) + exp (ACT) for ib
                    nmax = []
                    for c in range(NJC):
                        nm = st.tile([P, 1], F32, tag=f"nm{c}", name=f"nm{c}")
                        nc.vector.reduce_max(nm[:], cur[c][:], axis=AX, negate=True)
                        nmax.append(nm)
                    nm01 = st.tile([P, 1], F32, tag="nm01")
                    nc.vector.tensor_tensor(nm01[:], nmax[0][:], nmax[1][:], op=ALU.min)
                    nm23 = st.tile([P, 1], F32, tag="nm23")
                    nc.vector.tensor_tensor(nm23[:], nmax[2][:], nmax[3][:], op=ALU.min)
                    nmall = st.tile([P, 1], F32, tag="nmall")
                    nc.vector.tensor_tensor(nmall[:], nm01[:], nm23[:], op=ALU.min)

                    w_sb = cs.tile([P, NK], BF16, tag="w")
                    dcs = []
                    for c in range(NJC):
                        dc = st.tile([P, 1], F32, tag=f"dc{c}", name=f"dc{c}")
                        nc.scalar.activation(
                            w_sb[:, c * JC : (c + 1) * JC],
                            cur[c][:],
                            EXP,
                            bias=nmall[:],
                            scale=1.0,
                            accum_out=dc[:],
                        )
                        dcs.append(dc)
                    d01 = st.tile([P, 1], F32, tag="d01")
                    nc.vector.tensor_tensor(d01[:], dcs[0][:], dcs[1][:], op=ALU.add)
                    d23 = st.tile([P, 1], F32, tag="d23")
                    nc.vector.tensor_tensor(d23[:], dcs[2][:], dcs[3][:], op=ALU.add)
                    den = st.tile([P, 1], F32, tag="den")
                    nc.vector.tensor_tensor(den[:], d01[:], d23[:], op=ALU.add)
                    rinv = st.tile([P, 1], F32, tag="rinv")
                    nc.vector.reciprocal(rinv[:], den[:])

                    # --- next block's scores keep the PE busy during exp(ib)
                    if ib + 1 < NIB:
                        sch = emit_scores(ib + 1)

                    # --- transpose exp weights (bf16, 1 cyc/col); bf16 PSUM
                    # tiles pack 8 transposes per bank and give 2x DVE copies
                    wT = cs.tile([P, NK], BF16, tag="wT")
                    for g in range(2):
                        pt = psT.tile([P, 8 * P], BF16, tag="pt")
                        for jj in range(8):
                            js = 8 * g + jj
                            nc.tensor.transpose(
                                pt[:, jj * P : (jj + 1) * P],
                                w_sb[:, js * P : (js + 1) * P],
                                ident_b[:],
                            )
                        nc.vector.tensor_copy(wT[:, g * 8 * P : (g + 1) * 8 * P], pt[:])

                    # --- output matmul
                    po = psO.tile([P, D], F32, tag="po")
                    for js in range(NJB):
                        nc.tensor.matmul(
                            po[:],
                            wT[:, js * P : (js + 1) * P],
                            vp[js][:],
                            start=(js == 0),
                            stop=(js == NJB - 1),
                        )
                    ob = cs.tile([P, D], F32, tag="ob")
                    nc.vector.tensor_scalar_mul(ob[:], po[:], rinv[:])
                    nc.sync.dma_start(out_d[ib * P : (ib + 1) * P, :], ob[:])

    nc.compile()
    return nc


def _get_nc():
    if "nc" not in _CACHE:
        _CACHE["nc"] = _build()
    return _CACHE["nc"]


def kernel(query, key, value, Wq, Wk, Wv, _trace=False):
    query = np.asarray(query, dtype=np.float32)
    key = np.asarray(key, dtype=np.float32)
    value = np.asarray(value, dtype=np.float32)
    import ml_dtypes

    qT = np.ascontiguousarray(query.transpose(0, 2, 1))
    kT = np.ascontiguousarray(key.transpose(0, 2, 1))
    vT = np.ascontiguousarray(value.transpose(0, 2, 1).astype(ml_dtypes.bfloat16))
    wqN = np.ascontiguousarray(np.asarray(Wq, dtype=np.float32))
    wkN = np.ascontiguousarray(np.asarray(Wk, dtype=np.float32))
    wvT = np.ascontiguousarray(np.asarray(Wv, dtype=np.float32).T.astype(ml_dtypes.bfloat16))

    nc = _get_nc()
    in_maps = [
        {
            "qT": qT[b],
            "kT": kT[b],
            "vT": vT[b],
            "wqN": wqN,
            "wkN": wkN,
            "wvT": wvT,
        }
        for b in range(B)
    ]
    res = run_bass_kernel_spmd(nc, in_maps, list(range(B)), trace=_trace)
    out = np.stack([res.results[b]["out"] for b in range(B)]).astype(np.float32)
    if _trace:
        _CACHE["last_result"] = res
    return out


# revision 17
# speedup vs baseline: 1.0728x; 1.0629x over previous
"""CrossAttention TRN2 Bass kernel.

Problem: out[b] = softmax((q[b] @ Wq.T) @ (k[b] @ Wk.T).T) @ (v[b] @ Wv.T)
  q/k/v: [8, 2048, 512] f32, Wq/Wk/Wv: [512, 512] f32.

Sharding: data-parallel over batch -- core b computes batch b entirely.

Key optimizations vs the reference structure:
  * All operand transposes happen on the HOST (numpy) -- the device receives
    qT/kT/vT [D, N] and WqT/WkT/WvT [D, D], so the PE spends zero cycles
    transposing inputs.
  * Every projection / score matmul runs in f32r (fp32 bits, PE rounds
    operands to ~11-bit mantissa, 1 cycle/col vs fp32's 4).  Simulated
    end-to-end rel-err of this scheme is ~1.5e-2 against the 2e-2 gate.
  * Softmax weights are written as bf16 by the ACT exp, transposed on the PE
    at 1 cycle/col (vs 2 for fp32), and the output matmul runs bf16xbf16.
  * Phase C is software-pipelined: scores(ib+1) is issued to the PE between
    exp(ib) (ACT) and the weight-transposes(ib), so the PE never waits on
    the softmax statistics chain.

Per-core PE budget @2.4GHz: k'/q' proj 66k cyc, v' proj 33k, scores 131k,
w transposes 33k, output 131k  ->  ~165us + DMA lead-in.
"""
import sys

if "/opt/trn_rl_repo" not in sys.path:
    sys.path.insert(0, "/opt/trn_rl_repo")

import numpy as np

import concourse.bacc as bacc
import concourse.mybir as mybir
import concourse.tile as tile
from concourse.bass_utils import run_bass_kernel_spmd
from concourse.masks import make_identity

F32 = mybir.dt.float32
F32R = mybir.dt.float32r
BF16 = mybir.dt.bfloat16
AX = mybir.AxisListType.X
ALU = mybir.AluOpType
EXP = mybir.ActivationFunctionType.Exp

B, NQ, NK, D = 8, 2048, 2048, 512
P = 128
NDB = D // P    # feature blocks (4)
NIB = NQ // P   # query row blocks (16)
NJB = NK // P   # key row blocks (16)
JC = 512        # scores j-chunk width (one fp32 PSUM bank)
NJC = NK // JC  # 4

_CACHE = {}


def _build():
    nc = bacc.Bacc("TRN2", target_bir_lowering=False)
    qT_d = nc.dram_tensor("qT", [D, NQ], F32R, kind="ExternalInput")
    kT_d = nc.dram_tensor("kT", [D, NK], F32R, kind="ExternalInput")
    # v path is bf16 end-to-end: v' only needs ~8 bits (it lands in bf16
    # anyway) and halving vT's bytes relieves the shared DMA engine
    vT_d = nc.dram_tensor("vT", [D, NK], BF16, kind="ExternalInput")
    w_d = {
        # wq/wk NATIVE [e, d] (feed MT = Wk^T Wq contraction over e);
        # wv transposed [d, d'] (rhs of the v' projection)
        "wq": nc.dram_tensor("wqN", [D, D], F32R, kind="ExternalInput"),
        "wk": nc.dram_tensor("wkN", [D, D], F32R, kind="ExternalInput"),
        "wv": nc.dram_tensor("wvT", [D, D], BF16, kind="ExternalInput"),
    }
    out_d = nc.dram_tensor("out", [NQ, D], F32, kind="ExternalOutput")

    with tile.TileContext(nc) as tc:
        with (
            tc.tile_pool(name="persist", bufs=1) as pp,
            tc.tile_pool(name="cs", bufs=2) as cs,
            tc.tile_pool(name="stat", bufs=2) as st,
            tc.tile_pool(name="psS", bufs=5, space="PSUM") as psS,
        ):
            ident_b = pp.tile([P, P], BF16, tag="ident_b")
            make_identity(nc, ident_b[:])

            # persistent across phase C: raw qT chunks (scores lhsT), the
            # folded Mk = (Wq^T Wk) k^T (scores rhs), and v'
            qt = [pp.tile([P, NDB, JC], F32R, tag=f"qt{c}", name=f"qt{c}") for c in range(NJC)]
            mk = [pp.tile([P, NK], F32R, tag=f"mk{db}", name=f"mk{db}") for db in range(NDB)]
            vp = [pp.tile([P, D], BF16, tag=f"vp{jb}", name=f"vp{jb}") for jb in range(NJB)]

            def emit_scores(ib):
                qtile = qt[ib // 4]
                io = (ib % 4) * P
                chunks = []
                for c in range(NJC):
                    jsl = slice(c * JC, (c + 1) * JC)
                    sc_ = psS.tile([P, JC], F32, tag="sc")
                    for b1 in range(NDB):
                        nc.tensor.matmul(
                            sc_[:],
                            qtile[:, b1, io : io + P],
                            mk[b1][:, jsl],
                            start=(b1 == 0),
                            stop=(b1 == NDB - 1),
                        )
                    chunks.append(sc_)
                return chunks

            # ---------------- Phase B (PE order: MT, Mk, S0, v')
            # scores = q' k'^T = q (Wq^T Wk) k^T: fold the two projection
            # weights into M once (tiny), apply M to k^T only, and feed raw
            # qT as the scores stationary -- deletes the whole q' projection.
            with (
                tc.tile_pool(name="wp", bufs=1) as wp,
                tc.tile_pool(name="xp", bufs=1) as xp,
                tc.tile_pool(name="psP", bufs=3, space="PSUM") as psP,
            ):
                # All DMA issue up front; the issuing engine queues are
                # in-order, so no compute may sit ahead of a dma_start.
                # Critical path (Wk, Wq, kT, qT) on the SP queue; the bf16
                # v path on the ACT queue drains early.
                def wtile(wname, dtype, eng):
                    t = wp.tile([P, NDB, D], dtype, tag=f"wt_{wname}", name=f"wt_{wname}")
                    eng.dma_start(t[:], w_d[wname].rearrange("(a p) e -> p a e", p=P))
                    return t

                def load_x(xd, tiles, eng):
                    xre = xd.rearrange("(a p) n -> p a n", p=P)
                    for c in range(NJC):
                        eng.dma_start(tiles[c][:], xre[:, :, c * JC : (c + 1) * JC])
                    return tiles

                wt = {}
                wt["wk"] = wtile("wk", F32R, nc.sync)
                wt["wq"] = wtile("wq", F32R, nc.sync)
                kt = load_x(kT_d, [xp.tile([P, NDB, JC], F32R, tag=f"kt{c}", name=f"kt{c}") for c in range(NJC)], nc.sync)
                load_x(qT_d, qt, nc.sync)
                wt["wv"] = wtile("wv", BF16, nc.scalar)
                vt = load_x(vT_d, [xp.tile([P, NDB, JC], BF16, tag=f"vt{c}", name=f"vt{c}") for c in range(NJC)], nc.scalar)

                # MT[d2, d1] = sum_e Wk[e, d2] Wq[e, d1]  (16 matmuls)
                mtt = [wp.tile([P, D], F32R, tag=f"mtt{b2}", name=f"mtt{b2}") for b2 in range(NDB)]
                for b2 in range(NDB):
                    pm = psP.tile([P, D], F32, tag="pm")
                    for a in range(NDB):
                        nc.tensor.matmul(
                            pm[:],
                            wt["wk"][:, a, b2 * P : (b2 + 1) * P],
                            wt["wq"][:, a, :],
                            start=(a == 0),
                            stop=(a == NDB - 1),
                        )
                    nc.vector.tensor_copy(mtt[b2][:], pm[:])

                # Mk[d1, j] = sum_d2 MT[d2, d1] kT[d2, j]  (64 matmuls)
                for c in range(NJC):
                    sl = slice(c * JC, (c + 1) * JC)
                    for b1 in range(NDB):
                        pm = psP.tile([P, JC], F32, tag="pm")
                        for b2 in range(NDB):
                            nc.tensor.matmul(
                                pm[:],
                                mtt[b2][:, b1 * P : (b1 + 1) * P],
                                kt[c][:, b2, :],
                                start=(b2 == 0),
                                stop=(b2 == NDB - 1),
                            )
                        nc.vector.tensor_copy(mk[b1][:, sl], pm[:])

                # first scores block issued here: its softmax stats hide
                # entirely under the v' projection that follows on the PE
                sch = emit_scores(0)

                # v' projection: v'[j, d'] = sum_d vT[d, j] WvT[d, d']
                for jb in range(NJB):
                    pm = psP.tile([P, D], F32, tag="pm")
                    for db in range(NDB):
                        nc.tensor.matmul(
                            pm[:],
                            vt[jb // 4][:, db, (jb % 4) * P : (jb % 4 + 1) * P],
                            wt["wv"][:, db, :],
                            start=(db == 0),
                            stop=(db == NDB - 1),
                        )
                    nc.vector.tensor_copy(vp[jb][:], pm[:])

            # ---------------- Phase C: attention, pipelined over query blocks
            with (
                tc.tile_pool(name="psT", bufs=2, space="PSUM") as psT,
                tc.tile_pool(name="psO", bufs=1, space="PSUM") as psO,
            ):
                for ib in range(NIB):
                    cur = sch
                    # --- softmax stats (DVE) + exp (ACT) for ib
                    nmax = []
                    for c in range(NJC):
                        nm = st.tile([P, 1], F32, tag=f"nm{c}", name=f"nm{c}")
                        nc.vector.reduce_max(nm[:], cur[c][:], axis=AX, negate=True)
                        nmax.append(nm)
                    nm01 = st.tile([P, 1], F32, tag="nm01")
                    nc.vector.tensor_tensor(nm01[:], nmax[0][:], nmax[1][:], op=ALU.min)
                    nm23 = st.tile([P, 1], F32, tag="nm23")
                    nc.vector.tensor_tensor(nm23[:], nmax[2][:], nmax[3][:], op=ALU.min)
                    nmall = st.tile([P, 1], F32, tag="nmall")
                    nc.vector.tensor_tensor(nmall[:], nm01[:], nm23[:], op=ALU.min)

                    w_sb = cs.tile([P, NK], BF16, tag="w")
                    dcs = []
                    for c in range(NJC):
                        dc = st.tile([P, 1], F32, tag=f"dc{c}", name=f"dc{c}")
                        nc.scalar.activation(
                            w_sb[:, c * JC : (c + 1) * JC],
                            cur[c][:],
                            EXP,
                            bias=nmall[:],
                            scale=1.0,
                            accum_out=dc[:],
                        )
                        dcs.append(dc)
                    d01 = st.tile([P, 1], F32, tag="d01")
                    nc.vector.tensor_tensor(d01[:], dcs[0][:], dcs[1][:], op=ALU.add)
                    d23 = st.tile([P, 1], F32, tag="d23")
                    nc.vector.tensor_tensor(d23[:], dcs[2][:], dcs[3][:], op=ALU.add)
                    den = st.tile([P, 1], F32, tag="den")
                    nc.vector.tensor_tensor(den[:], d01[:], d23[:], op=ALU.add)
                    rinv = st.tile([P, 1], F32, tag="rinv")
                    nc.vector.reciprocal(rinv[:], den[:])

                    # --- next block's scores keep the PE busy during exp(ib)
                    if ib + 1 < NIB:
                        sch = emit_scores(ib + 1)

                    # --- transpose exp weights (bf16, 1 cyc/col); bf16 PSUM
                    # tiles pack 8 transposes per bank and give 2x DVE copies
                    wT = cs.tile([P, NK], BF16, tag="wT")
                    for g in range(2):
                        pt = psT.tile([P, 8 * P], BF16, tag="pt")
                        for jj in range(8):
                            js = 8 * g + jj
                            nc.tensor.transpose(
                                pt[:, jj * P : (jj + 1) * P],
                                w_sb[:, js * P : (js + 1) * P],
                                ident_b[:],
                            )
                        nc.vector.tensor_copy(wT[:, g * 8 * P : (g + 1) * 8 * P], pt[:])

                    # --- output matmul
                    po = psO.tile([P, D], F32, tag="po")
                    for js in range(NJB):
                        nc.tensor.matmul(
                            po[:],
                            wT[:, js * P : (js + 1) * P],
                            vp[js][:],
                            start=(js == 0),
                            stop=(js == NJB - 1),
                        )
                    ob = cs.tile([P, D], F32, tag="ob")
                    nc.vector.tensor_scalar_mul(ob[:], po[:], rinv[:])
                    nc.sync.dma_start(out_d[ib * P : (ib + 1) * P, :], ob[:])

    nc.compile()
    return nc


def _get_nc():
    if "nc" not in _CACHE:
        _CACHE["nc"] = _build()
    return _CACHE["nc"]


def kernel(query, key, value, Wq, Wk, Wv, _trace=False):
    query = np.asarray(query, dtype=np.float32)
    key = np.asarray(key, dtype=np.float32)
    value = np.asarray(value, dtype=np.float32)
    import ml_dtypes

    qT = np.ascontiguousarray(query.transpose(0, 2, 1))
    kT = np.ascontiguousarray(key.transpose(0, 2, 1))
    vT = np.ascontiguousarray(value.transpose(0, 2, 1).astype(ml_dtypes.bfloat16))
    wqN = np.ascontiguousarray(np.asarray(Wq, dtype=np.float32))
    wkN = np.ascontiguousarray(np.asarray(Wk, dtype=np.float32))
    wvT = np.ascontiguousarray(np.asarray(Wv, dtype=np.float32).T.astype(ml_dtypes.bfloat16))

    nc = _get_nc()
    in_maps = [
        {
            "qT": qT[b],
            "kT": kT[b],
            "vT": vT[b],
            "wqN": wqN,
            "wkN": wkN,
            "wvT": wvT,
        }
        for b in range(B)
    ]
    res = run_bass_kernel_spmd(nc, in_maps, list(range(B)), trace=_trace)
    out = np.stack([res.results[b]["out"] for b in range(B)]).astype(np.float32)
    if _trace:
        _CACHE["last_result"] = res
    return out


# revision 18
# speedup vs baseline: 1.0743x; 1.0014x over previous
"""CrossAttention TRN2 Bass kernel.

Problem: out[b] = softmax((q[b] @ Wq.T) @ (k[b] @ Wk.T).T) @ (v[b] @ Wv.T)
  q/k/v: [8, 2048, 512] f32, Wq/Wk/Wv: [512, 512] f32.

Sharding: data-parallel over batch -- core b computes batch b entirely.

Key optimizations vs the reference structure:
  * All operand transposes happen on the HOST (numpy) -- the device receives
    qT/kT [D, N] f32r, vT [D, N] bf16, Wq/Wk native + WvT; the PE spends
    zero cycles transposing inputs.
  * Weight fold: scores = q (Wq^T Wk) k^T.  M^T = Wk^T Wq is computed once
    (16 matmuls), applied to kT only (Mk, 64 matmuls), and raw qT is the
    scores stationary -- the entire q' projection is deleted.
  * Every projection / score matmul runs in f32r (fp32 bits, PE rounds
    operands to ~11-bit mantissa, 1 cycle/col vs fp32's 4).  Measured
    end-to-end rel-err 6.0e-3 against the 2e-2 gate.
  * Softmax weights are written as bf16 by the ACT exp, transposed on the PE
    at 1 cycle/col into bf16 PSUM (8 transposes/bank, 2x DVE copies), and
    the output matmul runs bf16xbf16.  The whole v path is bf16.
  * Software pipelining: scores(0) is issued between Mk and v' so its
    softmax stats hide under the v' projection; in steady state scores(ib+1)
    is issued between exp(ib) (ACT) and the weight-transposes(ib).  PSUM:
    5 score banks + 2 transpose + 1 output accumulator (+3 phase-B, scoped).
  * DMA: per-chunk tiles for chunk-granular deps; critical path (Wk, Wq,
    kT, qT in consume order) on the SP hwdge queue, bf16 v path on the ACT
    queue; no compute is ever queued ahead of a dma_start on those engines.

Per-core PE budget @2.4GHz: MT+Mk 20k cyc, v' 33k, scores 131k, w
transposes 33k, output 131k -> ~145us busy (measured ~98.5% PE occupancy),
~192us end-to-end including the ~20us fixed runtime bring-up and teardown.
"""
import sys

if "/opt/trn_rl_repo" not in sys.path:
    sys.path.insert(0, "/opt/trn_rl_repo")

import numpy as np

import concourse.bacc as bacc
import concourse.mybir as mybir
import concourse.tile as tile
from concourse.bass_utils import run_bass_kernel_spmd
from concourse.masks import make_identity

F32 = mybir.dt.float32
F32R = mybir.dt.float32r
BF16 = mybir.dt.bfloat16
AX = mybir.AxisListType.X
ALU = mybir.AluOpType
EXP = mybir.ActivationFunctionType.Exp

B, NQ, NK, D = 8, 2048, 2048, 512
P = 128
NDB = D // P    # feature blocks (4)
NIB = NQ // P   # query row blocks (16)
NJB = NK // P   # key row blocks (16)
JC = 512        # scores j-chunk width (one fp32 PSUM bank)
NJC = NK // JC  # 4

_CACHE = {}


def _build():
    nc = bacc.Bacc("TRN2", target_bir_lowering=False)
    qT_d = nc.dram_tensor("qT", [D, NQ], F32R, kind="ExternalInput")
    kT_d = nc.dram_tensor("kT", [D, NK], F32R, kind="ExternalInput")
    # v path is bf16 end-to-end: v' only needs ~8 bits (it lands in bf16
    # anyway) and halving vT's bytes relieves the shared DMA engine
    vT_d = nc.dram_tensor("vT", [D, NK], BF16, kind="ExternalInput")
    w_d = {
        # wq/wk NATIVE [e, d] (feed MT = Wk^T Wq contraction over e);
        # wv transposed [d, d'] (rhs of the v' projection)
        "wq": nc.dram_tensor("wqN", [D, D], F32R, kind="ExternalInput"),
        "wk": nc.dram_tensor("wkN", [D, D], F32R, kind="ExternalInput"),
        "wv": nc.dram_tensor("wvT", [D, D], BF16, kind="ExternalInput"),
    }
    out_d = nc.dram_tensor("out", [NQ, D], F32, kind="ExternalOutput")

    with tile.TileContext(nc) as tc:
        with (
            tc.tile_pool(name="persist", bufs=1) as pp,
            tc.tile_pool(name="cs", bufs=2) as cs,
            tc.tile_pool(name="stat", bufs=2) as st,
            tc.tile_pool(name="psS", bufs=5, space="PSUM") as psS,
        ):
            ident_b = pp.tile([P, P], BF16, tag="ident_b")
            make_identity(nc, ident_b[:])

            # persistent across phase C: raw qT chunks (scores lhsT), the
            # folded Mk = (Wq^T Wk) k^T (scores rhs), and v'
            qt = [pp.tile([P, NDB, JC], F32R, tag=f"qt{c}", name=f"qt{c}") for c in range(NJC)]
            mk = [pp.tile([P, NK], F32R, tag=f"mk{db}", name=f"mk{db}") for db in range(NDB)]
            vp = [pp.tile([P, D], BF16, tag=f"vp{jb}", name=f"vp{jb}") for jb in range(NJB)]

            def emit_scores(ib):
                qtile = qt[ib // 4]
                io = (ib % 4) * P
                chunks = []
                for c in range(NJC):
                    jsl = slice(c * JC, (c + 1) * JC)
                    sc_ = psS.tile([P, JC], F32, tag="sc")
                    for b1 in range(NDB):
                        nc.tensor.matmul(
                            sc_[:],
                            qtile[:, b1, io : io + P],
                            mk[b1][:, jsl],
                            start=(b1 == 0),
                            stop=(b1 == NDB - 1),
                        )
                    chunks.append(sc_)
                return chunks

            # ---------------- Phase B (PE order: MT, Mk, S0, v')
            # scores = q' k'^T = q (Wq^T Wk) k^T: fold the two projection
            # weights into M once (tiny), apply M to k^T only, and feed raw
            # qT as the scores stationary -- deletes the whole q' projection.
            with (
                tc.tile_pool(name="wp", bufs=1) as wp,
                tc.tile_pool(name="xp", bufs=1) as xp,
                tc.tile_pool(name="psP", bufs=3, space="PSUM") as psP,
            ):
                # All DMA issue up front; the issuing engine queues are
                # in-order, so no compute may sit ahead of a dma_start.
                # Critical path (Wk, Wq, kT, qT) on the SP queue; the bf16
                # v path on the ACT queue drains early.
                def wtile(wname, dtype, eng):
                    t = wp.tile([P, NDB, D], dtype, tag=f"wt_{wname}", name=f"wt_{wname}")
                    eng.dma_start(t[:], w_d[wname].rearrange("(a p) e -> p a e", p=P))
                    return t

                def load_x(xd, tiles, eng):
                    xre = xd.rearrange("(a p) n -> p a n", p=P)
                    for c in range(NJC):
                        eng.dma_start(tiles[c][:], xre[:, :, c * JC : (c + 1) * JC])
                    return tiles

                wt = {}
                wt["wk"] = wtile("wk", F32R, nc.sync)
                wt["wq"] = wtile("wq", F32R, nc.sync)
                kt = load_x(kT_d, [xp.tile([P, NDB, JC], F32R, tag=f"kt{c}", name=f"kt{c}") for c in range(NJC)], nc.sync)
                load_x(qT_d, qt, nc.sync)
                wt["wv"] = wtile("wv", BF16, nc.scalar)
                vt = load_x(vT_d, [xp.tile([P, NDB, JC], BF16, tag=f"vt{c}", name=f"vt{c}") for c in range(NJC)], nc.scalar)

                # MT[d2, d1] = sum_e Wk[e, d2] Wq[e, d1]  (16 matmuls)
                mtt = [wp.tile([P, D], F32R, tag=f"mtt{b2}", name=f"mtt{b2}") for b2 in range(NDB)]
                for b2 in range(NDB):
                    pm = psP.tile([P, D], F32, tag="pm")
                    for a in range(NDB):
                        nc.tensor.matmul(
                            pm[:],
                            wt["wk"][:, a, b2 * P : (b2 + 1) * P],
                            wt["wq"][:, a, :],
                            start=(a == 0),
                            stop=(a == NDB - 1),
                        )
                    nc.vector.tensor_copy(mtt[b2][:], pm[:])

                # Mk[d1, j] = sum_d2 MT[d2, d1] kT[d2, j]  (64 matmuls)
                for c in range(NJC):
                    sl = slice(c * JC, (c + 1) * JC)
                    for b1 in range(NDB):
                        pm = psP.tile([P, JC], F32, tag="pm")
                        for b2 in range(NDB):
                            nc.tensor.matmul(
                                pm[:],
                                mtt[b2][:, b1 * P : (b1 + 1) * P],
                                kt[c][:, b2, :],
                                start=(b2 == 0),
                                stop=(b2 == NDB - 1),
                            )
                        nc.vector.tensor_copy(mk[b1][:, sl], pm[:])

                # first scores block issued here: its softmax stats hide
                # entirely under the v' projection that follows on the PE
                sch = emit_scores(0)

                # v' projection: v'[j, d'] = sum_d vT[d, j] WvT[d, d']
                for jb in range(NJB):
                    pm = psP.tile([P, D], F32, tag="pm")
                    for db in range(NDB):
                        nc.tensor.matmul(
                            pm[:],
                            vt[jb // 4][:, db, (jb % 4) * P : (jb % 4 + 1) * P],
                            wt["wv"][:, db, :],
                            start=(db == 0),
                            stop=(db == NDB - 1),
                        )
                    nc.vector.tensor_copy(vp[jb][:], pm[:])

            # ---------------- Phase C: attention, pipelined over query blocks
            with (
                tc.tile_pool(name="psT", bufs=2, space="PSUM") as psT,
                tc.tile_pool(name="psO", bufs=1, space="PSUM") as psO,
            ):
                for ib in range(NIB):
                    cur = sch
                    # --- softmax stats (DVE) + exp (ACT) for ib
                    nmax = []
                    for c in range(NJC):
                        nm = st.tile([P, 1], F32, tag=f"nm{c}", name=f"nm{c}")
                        nc.vector.reduce_max(nm[:], cur[c][:], axis=AX, negate=True)
                        nmax.append(nm)
                    nm01 = st.tile([P, 1], F32, tag="nm01")
                    nc.vector.tensor_tensor(nm01[:], nmax[0][:], nmax[1][:], op=ALU.min)
                    nm23 = st.tile([P, 1], F32, tag="nm23")
                    nc.vector.tensor_tensor(nm23[:], nmax[2][:], nmax[3][:], op=ALU.min)
                    nmall = st.tile([P, 1], F32, tag="nmall")
                    nc.vector.tensor_tensor(nmall[:], nm01[:], nm23[:], op=ALU.min)

                    w_sb = cs.tile([P, NK], BF16, tag="w")
                    dcs = []
                    for c in range(NJC):
                        dc = st.tile([P, 1], F32, tag=f"dc{c}", name=f"dc{c}")
                        nc.scalar.activation(
                            w_sb[:, c * JC : (c + 1) * JC],
                            cur[c][:],
                            EXP,
                            bias=nmall[:],
                            scale=1.0,
                            accum_out=dc[:],
                        )
                        dcs.append(dc)
                    d01 = st.tile([P, 1], F32, tag="d01")
                    nc.vector.tensor_tensor(d01[:], dcs[0][:], dcs[1][:], op=ALU.add)
                    d23 = st.tile([P, 1], F32, tag="d23")
                    nc.vector.tensor_tensor(d23[:], dcs[2][:], dcs[3][:], op=ALU.add)
                    den = st.tile([P, 1], F32, tag="den")
                    nc.vector.tensor_tensor(den[:], d01[:], d23[:], op=ALU.add)
                    rinv = st.tile([P, 1], F32, tag="rinv")
                    nc.vector.reciprocal(rinv[:], den[:])

                    # --- next block's scores keep the PE busy during exp(ib)
                    if ib + 1 < NIB:
                        sch = emit_scores(ib + 1)

                    # --- transpose exp weights (bf16, 1 cyc/col); bf16 PSUM
                    # tiles pack 8 transposes per bank and give 2x DVE copies
                    wT = cs.tile([P, NK], BF16, tag="wT")
                    for g in range(2):
                        pt = psT.tile([P, 8 * P], BF16, tag="pt")
                        for jj in range(8):
                            js = 8 * g + jj
                            nc.tensor.transpose(
                                pt[:, jj * P : (jj + 1) * P],
                                w_sb[:, js * P : (js + 1) * P],
                                ident_b[:],
                            )
                        nc.vector.tensor_copy(wT[:, g * 8 * P : (g + 1) * 8 * P], pt[:])

                    # --- output matmul
                    po = psO.tile([P, D], F32, tag="po")
                    for js in range(NJB):
                        nc.tensor.matmul(
                            po[:],
                            wT[:, js * P : (js + 1) * P],
                            vp[js][:],
                            start=(js == 0),
                            stop=(js == NJB - 1),
                        )
                    ob = cs.tile([P, D], F32, tag="ob")
                    nc.vector.tensor_scalar_mul(ob[:], po[:], rinv[:])
                    nc.sync.dma_start(out_d[ib * P : (ib + 1) * P, :], ob[:])

    nc.compile()
    return nc


def _get_nc():
    if "nc" not in _CACHE:
        _CACHE["nc"] = _build()
    return _CACHE["nc"]


def kernel(query, key, value, Wq, Wk, Wv, _trace=False):
    query = np.asarray(query, dtype=np.float32)
    key = np.asarray(key, dtype=np.float32)
    value = np.asarray(value, dtype=np.float32)
    import ml_dtypes

    qT = np.ascontiguousarray(query.transpose(0, 2, 1))
    kT = np.ascontiguousarray(key.transpose(0, 2, 1))
    vT = np.ascontiguousarray(value.transpose(0, 2, 1).astype(ml_dtypes.bfloat16))
    wqN = np.ascontiguousarray(np.asarray(Wq, dtype=np.float32))
    wkN = np.ascontiguousarray(np.asarray(Wk, dtype=np.float32))
    wvT = np.ascontiguousarray(np.asarray(Wv, dtype=np.float32).T.astype(ml_dtypes.bfloat16))

    nc = _get_nc()
    in_maps = [
        {
            "qT": qT[b],
            "kT": kT[b],
            "vT": vT[b],
            "wqN": wqN,
            "wkN": wkN,
            "wvT": wvT,
        }
        for b in range(B)
    ]
    res = run_bass_kernel_spmd(nc, in_maps, list(range(B)), trace=_trace)
    out = np.stack([res.results[b]["out"] for b in range(B)]).astype(np.float32)
    if _trace:
        _CACHE["last_result"] = res
    return out
